# revision 1
# baseline (speedup 1.0000x reference)
"""3-layer GAT on 8 Trainium2 NeuronCores.

Strategy (edge-parallel, dst-sharded):
  - Relabel nodes so that each of 8 cores owns an equal slice of
    destination nodes, grouped into 128-node bins balanced by in-degree
    (greedy). Every bin gets the same padded edge budget so the whole
    device program is static.
  - Per layer: each core computes h = act @ W and the attention dot
    products for its own node slice (dense matmuls), then an AllGather
    replicates the packed [h | a_src | a_dst] table to every core.
  - Each core processes its own edges (sorted by destination bin):
    indirect-DMA gathers of source rows, exp(leaky_relu(a_src+a_dst))
    on ACT/DVE, and a one-hot scatter matmul per 128-edge chunk that
    accumulates both the messages and the softmax denominators in PSUM.
    Normalization happens per destination node after accumulation
    (out = (sum ex*h) / denom), which is mathematically identical to
    normalizing per edge. The segment-max is skipped: attention logits
    for this model are O(1) so plain exp is numerically safe.
  - Pad edges point at a sentinel table row whose a_src is -200, so
    exp() underflows to ~0 and they contribute nothing.

The module builds and compiles the Bass program on first call (keyed by
input shapes) and reuses it afterwards.
"""

import sys

try:
    import concourse  # noqa: F401  (provided via PYTHONPATH on axon hosts)
except ImportError:
    sys.path.insert(0, "/opt/trn_rl_repo")

import heapq

import numpy as np

import concourse.bacc as bacc
import concourse.bass as bass
import concourse.mybir as mybir
import concourse.tile as tile
from concourse.masks import make_identity

P = 128
NCORES = 8
NH = 8          # heads (layers 0/1)
HC = 64         # channels per head
HD = NH * HC    # 512
D0 = 128        # input feature dim
OUTC = 40      # final classes
CO = 64        # padded layer-2 width
COL0 = HD + 2 * NH   # 528 packed row: h(512) | a_src(8) | a_dst(8)
COL2 = CO + 2        # 66  packed row: h2(64) | a_src2 | a_dst2
G = 8          # chunks per gather super-chunk
NEG = 0.2      # leaky relu slope
SENTV = -200.0

f32 = mybir.dt.float32
f32r = mybir.dt.float32r
i32 = mybir.dt.int32
AF = mybir.ActivationFunctionType
ALU = mybir.AluOpType


# ----------------------------------------------------------------- host prep

def _balance_bins(deg, nbins):
    """Assign each node to a 128-slot bin, balancing summed in-degree."""
    n = deg.shape[0]
    order = np.argsort(-deg, kind="stable")
    bin_of = np.empty(n, np.int32)
    slot_of = np.empty(n, np.int32)
    counts = np.zeros(nbins, np.int32)
    loads = np.zeros(nbins, np.int64)
    heap = [(0, b) for b in range(nbins)]
    heapq.heapify(heap)
    for node in order:
        while True:
            _, b = heapq.heappop(heap)
            if counts[b] < P:
                break
        bin_of[node] = b
        slot_of[node] = counts[b]
        counts[b] += 1
        loads[b] += deg[node]
        if counts[b] < P:
            heapq.heappush(heap, (int(loads[b]), b))
    return bin_of, slot_of, loads


def _prep(edge_index, n_nodes):
    src = np.asarray(edge_index[0], dtype=np.int64)
    dst = np.asarray(edge_index[1], dtype=np.int64)
    loop = np.arange(n_nodes, dtype=np.int64)
    src = np.concatenate([src, loop])
    dst = np.concatenate([dst, loop])

    deg = np.bincount(dst, minlength=n_nodes)
    bpc = -(-n_nodes // (P * NCORES))          # bins per core
    nbins = bpc * NCORES
    npad = nbins * P
    nloc = bpc * P
    sent = npad                                 # sentinel row index

    bin_of, slot_of, loads = _balance_bins(deg, nbins)
    new_id = bin_of.astype(np.int64) * P + slot_of

    cpt = int(-(-int(loads.max()) // P))        # chunks per tile
    ept = cpt * P                               # edge slots per tile
    ch = bpc * cpt                              # chunks per core
    nsup = -(-ch // G)

    e_src = new_id[src].astype(np.int32)
    e_dst = new_id[dst].astype(np.int32)
    e_bin = (e_dst >> 7).astype(np.int64)
    e_slot = (e_dst & 127).astype(np.float32)

    order_e = np.argsort(e_bin, kind="stable")
    starts = np.zeros(nbins + 1, np.int64)
    starts[1:] = np.cumsum(np.bincount(e_bin, minlength=nbins))

    src_arr = np.full((NCORES, ch, P), sent, np.int32)
    dstf_arr = np.zeros((NCORES, ch, P), np.float32)
    for b in range(nbins):
        es = order_e[starts[b]:starts[b + 1]]
        c, t = divmod(b, bpc)
        pos = t * ept + np.arange(es.shape[0])
        chunk = pos >> 7
        pp = pos & 127
        src_arr[c, chunk, pp] = e_src[es]
        dstf_arr[c, chunk, pp] = e_slot[es]

    def relayout(a):
        # [ch, P] -> flat in [super][partition][g] order (ragged tail)
        out = np.empty(ch * P, a.dtype)
        off = 0
        for s in range(nsup):
            g0 = s * G
            g1 = min(g0 + G, ch)
            blk = np.ascontiguousarray(a[g0:g1, :].T)
            out[off:off + blk.size] = blk.ravel()
            off += blk.size
        return out

    per_core = []
    for c in range(NCORES):
        per_core.append({
            "srcidx": relayout(src_arr[c]),
            "dstf": relayout(dstf_arr[c]),
        })

    dims = dict(n=n_nodes, bpc=bpc, nbins=nbins, npad=npad, nloc=nloc,
                sent=sent, cpt=cpt, ch=ch, nsup=nsup)
    return dims, per_core, new_id


def _block_diag_a(att_s, att_d):
    """[NH,HC]x2 -> [HD, 2*NH] block matrix for a = h @ A."""
    a = np.zeros((HD, 2 * NH), np.float32)
    r = np.arange(HD)
    h = r >> 6
    c = r & 63
    a[r, h] = att_s[h, c]
    a[r, NH + h] = att_d[h, c]
    return a


# ------------------------------------------------------------- device build

def _build(dims, use_f32r=False, single=False, use_bf16=True):
    npad, nloc, bpc = dims["npad"], dims["nloc"], dims["bpc"]
    cpt, ch, nsup = dims["cpt"], dims["ch"], dims["nsup"]

    nc = bacc.Bacc("TRN2", target_bir_lowering=False, debug=False,
                   enable_asserts=True,
                   num_devices=1 if single else NCORES)

    def mmdt(ap):
        return ap.bitcast(f32r) if use_f32r else ap

    # inputs
    xt_ap = nc.dram_tensor("xt", [D0, nloc], f32, kind="ExternalInput").ap()
    srcidx_ap = nc.dram_tensor("srcidx", [ch * P], i32, kind="ExternalInput").ap()
    dstf_ap = nc.dram_tensor("dstf", [ch * P], f32, kind="ExternalInput").ap()
    w0_ap = nc.dram_tensor("w0", [D0, HD], f32, kind="ExternalInput").ap()
    w1_ap = nc.dram_tensor("w1", [HD, HD], f32, kind="ExternalInput").ap()
    w2_ap = nc.dram_tensor("w2", [HD, CO], f32, kind="ExternalInput").ap()
    a0_ap = nc.dram_tensor("a0", [HD, 2 * NH], f32, kind="ExternalInput").ap()
    a1_ap = nc.dram_tensor("a1", [HD, 2 * NH], f32, kind="ExternalInput").ap()
    a2_ap = nc.dram_tensor("a2", [CO, 2], f32, kind="ExternalInput").ap()
    b0_ap = nc.dram_tensor("b0r", [1, HD], f32, kind="ExternalInput").ap()
    b1_ap = nc.dram_tensor("b1r", [1, HD], f32, kind="ExternalInput").ap()
    b2_ap = nc.dram_tensor("b2r", [1, CO], f32, kind="ExternalInput").ap()
    iota_ap = nc.dram_tensor("iota", [P, P], f32, kind="ExternalInput").ap()
    sent0_ap = nc.dram_tensor("sent0", [1, COL0], f32, kind="ExternalInput").ap()
    sent2_ap = nc.dram_tensor("sent2", [1, COL2], f32, kind="ExternalInput").ap()
    out_ap = nc.dram_tensor("out", [nloc, OUTC], f32, kind="ExternalOutput").ap()

    with tile.TileContext(nc) as tc:
        with tc.tile_pool(name="const", bufs=1) as cp, \
             tc.tile_pool(name="work", bufs=3) as sb, \
             tc.tile_pool(name="psum", bufs=2, space="PSUM") as ps, \
             tc.tile_pool(name="dram", bufs=1, space="DRAM") as dp:

            # ---------- persistent constants in SBUF
            iota_t = cp.tile([P, P], f32)
            nc.sync.dma_start(iota_t[:], iota_ap[:])
            ident_t = cp.tile([P, P], f32)
            make_identity(nc, ident_t[:])
            ones_t = cp.tile([1, P], f32)
            nc.gpsimd.memset(ones_t[:], 1.0)

            w0_t = cp.tile([P, HD], f32)
            nc.sync.dma_start(w0_t[:], w0_ap[:])
            w1_t = [cp.tile([P, HD], f32, name=f"w1c{k}", tag=f"w1_{k}")
                    for k in range(4)]
            for k in range(4):
                nc.sync.dma_start(w1_t[k][:], w1_ap[k * P:(k + 1) * P, :])
            a0c_t = [cp.tile([P, 2 * NH], f32, name=f"a0c{k}", tag=f"a0_{k}")
                     for k in range(4)]
            a1c_t = [cp.tile([P, 2 * NH], f32, name=f"a1c{k}", tag=f"a1_{k}")
                     for k in range(4)]
            for k in range(4):
                nc.sync.dma_start(a0c_t[k][:], a0_ap[k * P:(k + 1) * P, :])
                nc.sync.dma_start(a1c_t[k][:], a1_ap[k * P:(k + 1) * P, :])
            sent0_t = cp.tile([1, COL0], f32)
            nc.sync.dma_start(sent0_t[:], sent0_ap[:])
            sent2_t = cp.tile([1, COL2], f32)
            nc.sync.dma_start(sent2_t[:], sent2_ap[:])

            # W0A0 = W0 @ A0  (for layer-0 fused attention projections)
            w0a0_ps = ps.tile([P, 2 * NH], f32, tag="den", bufs=1)
            for k in range(4):
                w0t_k = sb.tile([P, P], f32, tag="tload")
                nc.sync.dma_start(
                    w0t_k[:], w0_ap[:, k * P:(k + 1) * P].rearrange("a b -> b a"))
                nc.tensor.matmul(out=w0a0_ps[:], lhsT=w0t_k[:], rhs=a0c_t[k][:],
                                 start=(k == 0), stop=(k == 3))
            w0a0_t = cp.tile([P, 2 * NH], f32)
            nc.scalar.activation(w0a0_t[:], w0a0_ps[:], AF.Copy)

            # W1A1 = W1 @ A1  (layer-1 fused attention projections)
            w1a1_t = [cp.tile([P, 2 * NH], f32, name=f"w1a1c{m}",
                              tag=f"w1a1_{m}") for m in range(4)]
            for m in range(4):
                wa_ps = ps.tile([P, 2 * NH], f32, tag="den", bufs=1)
                for k in range(4):
                    w1t_km = sb.tile([P, P], f32, tag="tload")
                    nc.sync.dma_start(
                        w1t_km[:],
                        w1_ap[m * P:(m + 1) * P, k * P:(k + 1) * P]
                        .rearrange("a b -> b a"))
                    nc.tensor.matmul(out=wa_ps[:], lhsT=w1t_km[:],
                                     rhs=a1c_t[k][:],
                                     start=(k == 0), stop=(k == 3))
                nc.scalar.activation(w1a1_t[m][:], wa_ps[:], AF.Copy)

            # W2eff = [W2 (64 cols, zero-padded) | W2@as2^T | W2@ad2^T]
            a2_t = cp.tile([CO, 2], f32)
            nc.sync.dma_start(a2_t[:], a2_ap[:])
            w2e_t = [cp.tile([P, COL2], f32, name=f"w2ec{m}", tag=f"w2e_{m}")
                     for m in range(4)]
            for m in range(4):
                nc.sync.dma_start(w2e_t[m][:, 0:CO], w2_ap[m * P:(m + 1) * P, :])
                w2t_m = sb.tile([CO, P], f32, tag="tload2")
                nc.sync.dma_start(
                    w2t_m[:], w2_ap[m * P:(m + 1) * P, :].rearrange("a b -> b a"))
                w2a_ps = ps.tile([P, 2], f32, tag="den", bufs=1)
                nc.tensor.matmul(out=w2a_ps[:], lhsT=w2t_m[:], rhs=a2_t[:],
                                 start=True, stop=True)
                nc.scalar.activation(w2e_t[m][:, CO:COL2], w2a_ps[:], AF.Copy)

            # bias tiles broadcast across partitions via K=1 matmul
            def bias_tile(b_ap, width, tag):
                row = sb.tile([1, width], f32, tag="brow")
                nc.sync.dma_start(row[:], b_ap[:])
                bps = ps.tile([P, width], f32, tag="agg", bufs=2)
                nc.tensor.matmul(out=bps[:], lhsT=ones_t[:], rhs=row[:],
                                 start=True, stop=True)
                bt = cp.tile([P, width], f32, name=tag, tag=tag)
                nc.scalar.activation(bt[:], bps[:], AF.Copy)
                return bt

            b0_t = bias_tile(b0_ap, HD, "b0t")
            b1_t = bias_tile(b1_ap, HD, "b1t")
            b2_t = bias_tile(b2_ap, CO, "b2t")

            # ---------- DRAM scratch
            ag0_in = dp.tile([nloc, COL0], f32)
            ag1_in = dp.tile([nloc, COL0], f32)
            ag2_in = dp.tile([nloc, COL2], f32)
            hcat0 = dp.tile([npad + 1, COL0], f32)
            hcat1 = dp.tile([npad + 1, COL0], f32)
            hcat2 = dp.tile([npad + 1, COL2], f32)
            actt1 = dp.tile([HD, nloc], f32)
            actt2 = dp.tile([HD, nloc], f32)

            # ---------- helpers
            def node_phase_l0():
                for t in range(bpc):
                    xt_t = sb.tile([P, P], f32, tag="lhs")
                    nc.sync.dma_start(xt_t[:], xt_ap[:, t * P:(t + 1) * P])
                    h_ps = ps.tile([P, HD], f32, tag="agg", bufs=2)
                    a_ps = ps.tile([P, 2 * NH], f32, tag="den", bufs=1)
                    nc.tensor.matmul(out=h_ps[:], lhsT=mmdt(xt_t[:]),
                                     rhs=mmdt(w0_t[:]), start=True, stop=True)
                    nc.tensor.matmul(out=a_ps[:], lhsT=xt_t[:], rhs=w0a0_t[:],
                                     start=True, stop=True)
                    cat = sb.tile([P, COL0], f32, tag="cat")
                    nc.scalar.activation(cat[:, :HD], h_ps[:], AF.Copy)
                    nc.vector.tensor_copy(cat[:, HD:COL0], a_ps[:])
                    nc.sync.dma_start(ag0_in[t * P:(t + 1) * P, :], cat[:])

            def node_phase(actt, w_chunks, a_chunks, ag_in, width, ncol):
                r = mmdt if width == HD else (lambda a: a)
                for t in range(bpc):
                    h_ps = ps.tile([P, width], f32, tag="agg", bufs=2)
                    a_ps = ps.tile([P, 2 * NH], f32, tag="den", bufs=1)
                    for k in range(4):
                        at_k = sb.tile([P, P], f32, tag="lhs")
                        nc.sync.dma_start(
                            at_k[:], actt[k * P:(k + 1) * P, t * P:(t + 1) * P])
                        nc.tensor.matmul(out=h_ps[:], lhsT=r(at_k[:]),
                                         rhs=r(w_chunks[k][:]),
                                         start=(k == 0), stop=(k == 3))
                        if a_chunks is not None:
                            nc.tensor.matmul(out=a_ps[:], lhsT=at_k[:],
                                             rhs=a_chunks[k][:],
                                             start=(k == 0), stop=(k == 3))
                    cat = sb.tile([P, ncol], f32, tag="cat")
                    nc.scalar.activation(cat[:, :width], h_ps[:], AF.Copy)
                    if a_chunks is not None:
                        nc.vector.tensor_copy(cat[:, width:width + 2 * NH], a_ps[:])
                    nc.sync.dma_start(ag_in[t * P:(t + 1) * P, :], cat[:])

            def allgather(ag_in, hcat_f, ncol, sent_t):
                if single:
                    # timeline-sim mode: stand-in copy, no collective
                    nc.sync.dma_start(hcat_f[0:nloc, :], ag_in[:])
                else:
                    nc.gpsimd.collective_compute(
                        "AllGather", ALU.bypass,
                        replica_groups=[list(range(NCORES))],
                        ins=[ag_in[:].opt()],
                        outs=[hcat_f[:npad, :].opt()],
                    )
                nc.sync.dma_start(hcat_f[npad:npad + 1, :], sent_t[:])

            def agg_layer(hcat_f, ag_in, colw, hwid, flush):
                """Edge aggregation: supers outer, chunks inner."""
                nhh = NH if hwid == HD else 1
                aoff = hwid + 2 * NH if hwid == HD else 0  # asrc col offset
                asrc_c = hwid
                adst_c = hwid + nhh
                state = {}  # tile -> (o_ps, d_ps, adst_t)
                for s in range(nsup):
                    g_s = min(G, ch - s * G)
                    off = s * G * P
                    idx_t = sb.tile([P, G], i32, tag="idx")
                    nc.sync.dma_start(
                        idx_t[:, :g_s],
                        srcidx_ap[off:off + P * g_s]
                        .rearrange("(p g) -> p g", g=g_s))
                    dstf_t = sb.tile([P, G], f32, tag="dstf")
                    nc.sync.dma_start(
                        dstf_t[:, :g_s],
                        dstf_ap[off:off + P * g_s]
                        .rearrange("(p g) -> p g", g=g_s))

                    mmdt2 = (mybir.dt.bfloat16 if use_bf16 else f32)
                    # one-hot P for all chunks of the super, then transpose
                    pm = sb.tile([P, G * P], mmdt2, tag="pm")
                    pmf = sb.tile([P, G * P], f32, tag="pmf")
                    nc.vector.tensor_tensor(
                        out=pmf[:, :g_s * P].rearrange("p (g j) -> p g j",
                                                       g=g_s),
                        in0=dstf_t[:, :g_s]
                            .rearrange("p (g one) -> p g one", one=1)
                            .broadcast_to([P, g_s, P]),
                        in1=iota_t[:].rearrange("p (one j) -> p one j", one=1)
                            .broadcast_to([P, g_s, P]),
                        op=ALU.is_equal,
                    )
                    if use_bf16:
                        nc.scalar.activation(pm[:, :g_s * P],
                                             pmf[:, :g_s * P], AF.Copy)
                    pt_ps = ps.tile([P, G * P], f32, tag="tr", bufs=2)
                    for g in range(g_s):
                        nc.tensor.transpose(out=pt_ps[:, g * P:(g + 1) * P],
                                            in_=pmf[:, g * P:(g + 1) * P],
                                            identity=ident_t[:])
                    pt_sb = sb.tile([P, G * P], f32, tag="pt")
                    nc.scalar.activation(pt_sb[:, :g_s * P],
                                         pt_ps[:, :g_s * P], AF.Copy)

                    rowg = sb.tile([P, G * colw], f32, tag="rowg", bufs=4)
                    ade_ps = ps.tile([P, G * nhh], f32, tag="ade", bufs=1)
                    for g in range(g_s):
                        chk = s * G + g
                        t, c = divmod(chk, cpt)
                        if c == 0:
                            o_ps = ps.tile([P, hwid], f32, tag="agg",
                                           name="o_ps", bufs=2)
                            d_ps = ps.tile([P, nhh], f32, tag="den",
                                           name="d_ps", bufs=1)
                            adst_t = sb.tile([P, NH], f32, tag="adst",
                                             name="adst_t")
                            nc.sync.dma_start(
                                adst_t[:, :nhh],
                                ag_in[t * P:(t + 1) * P, adst_c:adst_c + nhh])
                            state[t] = (o_ps, d_ps, adst_t)
                        # gather full rows for this chunk
                        nc.gpsimd.indirect_dma_start(
                            out=rowg[:, g * colw:(g + 1) * colw],
                            out_offset=None,
                            in_=hcat_f[:, :],
                            in_offset=bass.IndirectOffsetOnAxis(
                                ap=idx_t[:, g:g + 1], axis=0),
                        )
                        # a_dst lookup for this chunk via transposed one-hot
                        nc.tensor.matmul(
                            out=ade_ps[:, g * nhh:(g + 1) * nhh],
                            lhsT=pt_sb[:, g * P:(g + 1) * P],
                            rhs=state[t][2][:, :nhh],
                            start=True, stop=True)

                    # batched attention math for the whole super
                    ne = g_s * nhh
                    rview = rowg[:, :g_s * colw].rearrange(
                        "p (g w) -> p g w", g=g_s)
                    ex = sb.tile([P, G * NH], f32, tag="ex")
                    tmp = sb.tile([P, G * NH], f32, tag="tmp")
                    nc.vector.tensor_tensor(
                        out=ex[:, :ne].rearrange("p (g h) -> p g h", g=g_s),
                        in0=rview[:, :, asrc_c:asrc_c + nhh],
                        in1=ade_ps[:, :ne].rearrange("p (g h) -> p g h",
                                                     g=g_s),
                        op=ALU.add)
                    nc.vector.tensor_scalar_mul(tmp[:, :ne], ex[:, :ne], NEG)
                    nc.vector.tensor_tensor(out=ex[:, :ne], in0=ex[:, :ne],
                                            in1=tmp[:, :ne], op=ALU.max)
                    nc.scalar.activation(ex[:, :ne], ex[:, :ne], AF.Exp)
                    sc = sb.tile([P, G * HD], mmdt2, tag="sc")
                    exd = sb.tile([P, G * NH], mmdt2, tag="exd")
                    if use_bf16:
                        nc.scalar.activation(exd[:, :ne], ex[:, :ne], AF.Copy)
                    nc.vector.tensor_tensor(
                        out=sc[:, :g_s * hwid].rearrange(
                            "p (g h c) -> p g h c", g=g_s, c=HC),
                        in0=rview[:, :, :hwid].rearrange(
                            "p g (h c) -> p g h c", c=HC),
                        in1=ex[:, :ne].rearrange(
                            "p (g h one) -> p g h one", g=g_s, one=1)
                            .broadcast_to([P, g_s, nhh, HC]),
                        op=ALU.mult)

                    # scatter matmuls + flushes
                    for g in range(g_s):
                        chk = s * G + g
                        t, c = divmod(chk, cpt)
                        o_ps, d_ps, _ = state[t]
                        den_lhs = pm if use_bf16 else pmf
                        den_rhs = exd if use_bf16 else ex
                        nc.tensor.matmul(
                            out=o_ps[:],
                            lhsT=mmdt(den_lhs[:, g * P:(g + 1) * P]),
                            rhs=mmdt(sc[:, g * hwid:(g + 1) * hwid]),
                            start=(c == 0), stop=(c == cpt - 1))
                        nc.tensor.matmul(
                            out=d_ps[:], lhsT=den_lhs[:, g * P:(g + 1) * P],
                            rhs=den_rhs[:, g * nhh:(g + 1) * nhh],
                            start=(c == 0), stop=(c == cpt - 1))
                        if c == cpt - 1:
                            flush(t, o_ps, d_ps)
                            del state[t]

            def flush_big(t, o_ps, d_ps, b_t, actt_next):
                den = sb.tile([P, NH], f32, tag="den_sb")
                nc.vector.tensor_scalar_add(den[:], d_ps[:], 1e-20)
                rden = sb.tile([P, NH], f32, tag="rden")
                nc.vector.reciprocal(rden[:], den[:])
                o_sb = sb.tile([P, HD], f32, tag="osb")
                nc.vector.tensor_tensor(
                    out=o_sb[:].rearrange("p (h c) -> p h c", c=HC),
                    in0=o_ps[:].rearrange("p (h c) -> p h c", c=HC),
                    in1=rden[:].rearrange("p (h one) -> p h one", one=1)
                        .broadcast_to([P, NH, HC]),
                    op=ALU.mult)
                nc.vector.tensor_tensor(out=o_sb[:], in0=o_sb[:], in1=b_t[:],
                                        op=ALU.add)
                nc.scalar.activation(o_sb[:], o_sb[:], AF.Relu)
                for k in range(4):
                    tr_ps = ps.tile([P, P], f32, tag="tr", bufs=2)
                    nc.tensor.transpose(out=tr_ps[:],
                                        in_=o_sb[:, k * P:(k + 1) * P],
                                        identity=ident_t[:])
                    tr_sb = sb.tile([P, P], f32, tag="trsb")
                    nc.scalar.activation(tr_sb[:], tr_ps[:], AF.Copy)
                    nc.sync.dma_start(
                        actt_next[k * P:(k + 1) * P, t * P:(t + 1) * P], tr_sb[:])

            def flush_l2(t, o_ps, d_ps):
                den = sb.tile([P, 1], f32, tag="den_sb")
                nc.vector.tensor_scalar_add(den[:], d_ps[:], 1e-20)
                rden = sb.tile([P, 1], f32, tag="rden")
                nc.vector.reciprocal(rden[:], den[:])
                o_sb = sb.tile([P, CO], f32, tag="osb")
                nc.vector.tensor_tensor(
                    out=o_sb[:], in0=o_ps[:],
                    in1=rden[:].broadcast_to([P, CO]), op=ALU.mult)
                nc.vector.tensor_tensor(out=o_sb[:], in0=o_sb[:], in1=b2_t[:],
                                        op=ALU.add)
                mx = sb.tile([P, 1], f32, tag="mx")
                nc.vector.tensor_reduce(out=mx[:], in_=o_sb[:, :OUTC],
                                        axis=mybir.AxisListType.X, op=ALU.max)
                t2 = sb.tile([P, OUTC], f32, tag="t2")
                nc.vector.tensor_tensor(out=t2[:], in0=o_sb[:, :OUTC],
                                        in1=mx[:].broadcast_to([P, OUTC]),
                                        op=ALU.subtract)
                exl = sb.tile([P, OUTC], f32, tag="exl")
                nc.scalar.activation(exl[:], t2[:], AF.Exp)
                sm = sb.tile([P, 1], f32, tag="sm")
                nc.vector.tensor_reduce(out=sm[:], in_=exl[:],
                                        axis=mybir.AxisListType.X, op=ALU.add)
                ls = sb.tile([P, 1], f32, tag="ls")
                nc.scalar.activation(ls[:], sm[:], AF.Ln)
                res = sb.tile([P, OUTC], f32, tag="res")
                nc.vector.tensor_tensor(out=res[:], in0=t2[:],
                                        in1=ls[:].broadcast_to([P, OUTC]),
                                        op=ALU.subtract)
                nc.sync.dma_start(out_ap[t * P:(t + 1) * P, :], res[:])

            # ---------- the program
            node_phase_l0()
            allgather(ag0_in, hcat0, COL0, sent0_t)
            agg_layer(hcat0, ag0_in, COL0, HD,
                      lambda t, o, d: flush_big(t, o, d, b0_t, actt1))
            node_phase(actt1, w1_t, w1a1_t, ag1_in, HD, COL0)
            allgather(ag1_in, hcat1, COL0, sent0_t)
            agg_layer(hcat1, ag1_in, COL0, HD,
                      lambda t, o, d: flush_big(t, o, d, b1_t, actt2))
            node_phase(actt2, w2e_t, None, ag2_in, COL2, COL2)
            allgather(ag2_in, hcat2, COL2, sent2_t)
            agg_layer(hcat2, ag2_in, COL2, CO, flush_l2)

    nc.compile()
    return nc


# ------------------------------------------------------------------ runners

_CACHE = {}


def _get_program(dims, use_f32r=False, use_bf16=True):
    key = (tuple(sorted(dims.items())), use_f32r, use_bf16)
    if key not in _CACHE:
        _CACHE[key] = _build(dims, use_f32r=use_f32r, use_bf16=use_bf16)
    return _CACHE[key]


def make_in_maps(x, W0, as0, ad0, b0, W1, as1, ad1, b1, W2, as2, ad2, b2,
                 dims, per_core, new_id):
    npad, nloc = dims["npad"], dims["nloc"]
    n = dims["n"]
    xp = np.zeros((npad, D0), np.float32)
    xp[new_id] = np.asarray(x, np.float32)

    w2p = np.zeros((HD, CO), np.float32)
    w2p[:, :OUTC] = W2
    a2c = np.zeros((CO, 2), np.float32)
    a2c[:OUTC, 0] = np.asarray(as2, np.float32)[0]
    a2c[:OUTC, 1] = np.asarray(ad2, np.float32)[0]
    b2p = np.zeros((1, CO), np.float32)
    b2p[0, :OUTC] = b2

    sent0 = np.zeros((1, COL0), np.float32)
    sent0[0, HD:] = SENTV
    sent2 = np.zeros((1, COL2), np.float32)
    sent2[0, CO:] = SENTV

    shared = {
        "w0": np.asarray(W0, np.float32),
        "w1": np.asarray(W1, np.float32),
        "w2": w2p,
        "a0": _block_diag_a(np.asarray(as0, np.float32),
                            np.asarray(ad0, np.float32)),
        "a1": _block_diag_a(np.asarray(as1, np.float32),
                            np.asarray(ad1, np.float32)),
        "a2": a2c,
        "b0r": np.asarray(b0, np.float32).reshape(1, HD),
        "b1r": np.asarray(b1, np.float32).reshape(1, HD),
        "b2r": b2p,
        "iota": np.tile(np.arange(P, dtype=np.float32)[None, :], (P, 1)),
        "sent0": sent0,
        "sent2": sent2,
    }
    in_maps = []
    for c in range(NCORES):
        m = dict(shared)
        m["xt"] = np.ascontiguousarray(xp[c * nloc:(c + 1) * nloc].T)
        m.update(per_core[c])
        in_maps.append(m)
    return in_maps


def assemble_output(results, dims, new_id):
    nloc, n = dims["nloc"], dims["n"]
    full = np.concatenate([results[c]["out"] for c in range(NCORES)], axis=0)
    return np.ascontiguousarray(full[new_id[:n]])


def kernel(x, edge_index, W0, as0, ad0, b0, W1, as1, ad1, b1,
           W2, as2, ad2, b2):
    from concourse import bass_utils

    n = x.shape[0]
    dims, per_core, new_id = _prep(np.asarray(edge_index), n)
    prog = _get_program(dims)
    in_maps = make_in_maps(x, W0, as0, ad0, b0, W1, as1, ad1, b1,
                           W2, as2, ad2, b2, dims, per_core, new_id)
    res = bass_utils.run_bass_kernel_spmd(prog, in_maps,
                                          core_ids=list(range(NCORES)))
    return assemble_output(res.results, dims, new_id)



# revision 46
# speedup vs baseline: 3.3032x; 3.3032x over previous
"""3-layer GAT on 8 Trainium2 NeuronCores.

Strategy (edge-parallel, dst-sharded):
  - Relabel nodes so each of 8 cores owns an equal slice of destination
    nodes, grouped into 128-node bins balanced by in-degree. Every bin
    gets the same padded edge budget (cpt chunks of 128 edges) so the
    whole device program is static.
  - Per layer: each core computes the packed bf16 row [h | a_src] for
    its own node slice (dense bf16 matmuls against host-prefused
    weight/attention matrices), then an AllGather replicates the table
    to every core (row stride 256B-aligned for the gather engine).
    a_dst stays core-local in SBUF.
  - Each core processes its own edges: batched dma_gather pulls up to
    6x128 source rows per SWDGE call (indices int16, so each tile's
    edges are pre-sorted by source row and split at a chunk boundary
    into a low half gathered from hcat[0:32768] and a high half from
    hcat[npad-32768:], rebasing the indices). Per-edge attention
    exp(leaky_relu(a_src+a_dst)) = max(e^l, e^(l/5)) runs on ACT/DVE;
    per 128-edge chunk a pair of one-hot scatter matmuls accumulates
    messages (N=512) and softmax denominators (N=8) in PSUM. The
    fp8 one-hot matrices (pm, and pt for the per-edge a_dst lookup)
    are host-precomputed, shared by all three layers, and streamed
    per destination tile in a single merged DMA.
  - Layer 2 folds its denominator into the scatter matmul via a
    constant-1 table channel.
  - Normalization happens per destination node at tile flush; the
    segment-max is skipped (logits are O(1) here, plain exp is safe).
    Pad edges have all-zero one-hot columns so they contribute exactly
    nothing. The next layer's node phase (PE transposes + dense
    matmuls + row assembly) is fused into each tile's flush, so layers
    pipeline without an activation round-trip through DRAM.

The module builds and compiles the Bass program on first call (keyed by
input shapes) and reuses it afterwards.
"""
import sys

try:
    import concourse  # noqa: F401  (provided via PYTHONPATH on axon hosts)
except ImportError:
    sys.path.insert(0, "/opt/trn_rl_repo")

import heapq

import ml_dtypes
import numpy as np

import concourse.bacc as bacc
import concourse.bass as bass
import concourse.mybir as mybir
import concourse.tile as tile
from concourse.masks import make_identity

P = 128
NCORES = 8
NH = 8          # heads (layers 0/1)
HC = 64         # channels per head
HD = NH * HC    # 512
D0 = 128        # input feature dim
OUTC = 40       # final classes
COL0 = HD + NH          # 520 used cols: h(512) | a_src(8)
ROW0 = 640              # hcat row stride (256B-multiple for dma_gather)
COL2 = OUTC + 3         # 43 cols: h2(40) | den-1s | a_src2 | a_dst2-stash
ROW2 = 128              # layer-2 hcat row stride
NEG = 0.2       # leaky relu slope
SENTV = -200.0

f32 = mybir.dt.float32
bf16 = mybir.dt.bfloat16
fp8 = mybir.dt.float8e4
i32 = mybir.dt.int32
i16 = mybir.dt.int16
AF = mybir.ActivationFunctionType
ALU = mybir.AluOpType

npbf16 = ml_dtypes.bfloat16
npfp8 = ml_dtypes.float8_e4m3fn


# ----------------------------------------------------------------- host prep

def _balance_bins(deg, nbins):
    """Assign each node to a 128-slot bin, balancing summed in-degree."""
    n = deg.shape[0]
    order = np.argsort(-deg, kind="stable")
    bin_of = np.empty(n, np.int32)
    slot_of = np.empty(n, np.int32)
    counts = np.zeros(nbins, np.int32)
    loads = np.zeros(nbins, np.int64)
    heap = [(0, b) for b in range(nbins)]
    heapq.heapify(heap)
    for node in order:
        while True:
            _, b = heapq.heappop(heap)
            if counts[b] < P:
                break
        bin_of[node] = b
        slot_of[node] = counts[b]
        counts[b] += 1
        loads[b] += deg[node]
        if counts[b] < P:
            heapq.heappush(heap, (int(loads[b]), b))
    return bin_of, slot_of, loads


def _prep(edge_index, n_nodes):
    src = np.asarray(edge_index[0], dtype=np.int64)
    dst = np.asarray(edge_index[1], dtype=np.int64)
    loop = np.arange(n_nodes, dtype=np.int64)
    src = np.concatenate([src, loop])
    dst = np.concatenate([dst, loop])

    deg = np.bincount(dst, minlength=n_nodes)
    bpc = -(-n_nodes // (P * NCORES))          # bins per core
    nbins = bpc * NCORES
    npad = nbins * P
    nloc = bpc * P

    bin_of, slot_of, loads = _balance_bins(deg, nbins)
    new_id = bin_of.astype(np.int64) * P + slot_of

    cpt = int(-(-int(loads.max()) // P))        # chunks per tile
    ept = cpt * P                               # edge slots per tile
    ch = bpc * cpt                              # chunks per core
    ca = cpt // 2                               # lo chunks per tile
    cb = cpt - ca                               # hi chunks per tile
    hi_base = npad - 32768                      # hi-view base row
    assert hi_base >= 0 and npad - hi_base <= 32768

    e_src = new_id[src].astype(np.int32)
    e_dst = new_id[dst].astype(np.int32)
    e_bin = (e_dst >> 7).astype(np.int64)
    e_slot = (e_dst & 127).astype(np.int32)

    order_e = np.argsort(e_bin, kind="stable")
    starts = np.zeros(nbins + 1, np.int64)
    starts[1:] = np.cumsum(np.bincount(e_bin, minlength=nbins))

    src_arr = np.full((NCORES, ch, P), -1, np.int32)
    slot_arr = np.zeros((NCORES, ch, P), np.int32)
    for b in range(nbins):
        es = order_e[starts[b]:starts[b + 1]]
        c, t = divmod(b, bpc)
        srcs = e_src[es]
        o = np.argsort(srcs, kind="stable")
        es, srcs = es[o], srcs[o]
        ne = es.shape[0]
        hi_min = int(np.searchsorted(srcs, hi_base))
        lo_max = int(np.searchsorted(srcs, 32768))
        cut = min(ca * P, lo_max)
        cut = max(cut, hi_min, ne - cb * P)
        assert hi_min <= cut <= min(ca * P, lo_max), (b, hi_min, lo_max, ne)
        # lo edges -> slots [0, cut), hi edges -> slots [ca*P, ca*P + ne-cut)
        pos = np.concatenate([np.arange(cut),
                              ca * P + np.arange(ne - cut)])
        chunk = (t * ept + pos) >> 7
        pp = pos & 127
        src_arr[c, chunk, pp] = e_src[es]
        slot_arr[c, chunk, pp] = e_slot[es]

    # call lists: J <= 8 per dma_gather call (the SWDGE ring holds 1024
    # descriptors; 9*128 overflows it). Big layers use smaller calls for
    # finer DMA interleaving; layer 2's gathers are tiny so fewer calls win.
    def mkcalls(jcap):
        calls = []
        for t in range(bpc):
            for side, s0, cnt in ((0, t * cpt, ca), (1, t * cpt + ca, cb)):
                off = 0
                while off < cnt:
                    j = min(jcap, cnt - off)
                    calls.append((side, s0 + off, j))
                    off += j
        return tuple(calls)

    def mkcalls_bal(jcap):
        calls = []
        for t in range(bpc):
            for side, s0, cnt in ((0, t * cpt, ca), (1, t * cpt + ca, cb)):
                nparts = -(-cnt // jcap)
                off = 0
                for k in range(nparts):
                    j = (cnt - off) // (nparts - k)
                    calls.append((side, s0 + off, j))
                    off += j
        return tuple(calls)

    calls = mkcalls(6)
    calls2 = mkcalls(8)

    # chunk side: lo for local chunk < ca
    cidx = np.arange(ch)
    is_hi = (cidx % cpt) >= ca                  # [ch]

    per_core = []
    pp = np.arange(P)
    rr = 16 + (pp % 16)                         # ucode reads partitions 16..31
    ss = pp // 16
    for c in range(NCORES):
        valid = src_arr[c] >= 0                 # [ch, P]
        sv = np.maximum(src_arr[c], 0)
        sv = sv - np.where(is_hi[:, None], hi_base, 0)
        sv = np.maximum(sv, 0)                  # pads on hi side -> row 0
        idx16 = np.zeros((P, ch * 8), np.int16)
        idx16[rr[None, :].repeat(ch, 0),
              cidx[:, None] * 8 + ss[None, :]] = sv.astype(np.int16)
        pmpt = np.zeros((P, ch * 2 * P), npfp8)
        pmr = pp[None, :].repeat(ch, 0)
        pmc = cidx[:, None] * 2 * P + slot_arr[c]
        pmpt[pmr[valid], pmc[valid]] = 1.0
        ptr = slot_arr[c]
        ptc = cidx[:, None] * 2 * P + P + pp[None, :]
        pmpt[ptr[valid], ptc[valid]] = 1.0
        per_core.append({"srcidx": idx16, "pmpt": pmpt})

    dims = dict(n=n_nodes, bpc=bpc, nbins=nbins, npad=npad, nloc=nloc,
                cpt=cpt, ch=ch, ca=ca, cb=cb, hi_base=hi_base, calls=calls,
                calls2=calls2)
    return dims, per_core, new_id


def _with_bias_flag(dims, b0, b1, b2):
    d = dict(dims)
    d["use_bias"] = bool(np.any(b0) or np.any(b1) or np.any(b2))
    return d


def _block_diag_a(att_s, att_d):
    """[NH,HC]x2 -> [HD, 2*NH] block matrix for a = h @ A."""
    a = np.zeros((HD, 2 * NH), np.float32)
    r = np.arange(HD)
    h = r >> 6
    c = r & 63
    a[r, h] = att_s[h, c]
    a[r, NH + h] = att_d[h, c]
    return a


def _dma_gather_raw(gp, out_ap, in_ap, idxs_ap, num_idxs, elem_size,
                    elem_step):
    """nc.gpsimd.dma_gather without the 256B *payload* restriction.

    The 256B rule is a transpose-mode restriction; the non-transpose ucode
    handles arbitrary payload bytes. The row *stride* (elem_step) must still
    be a multiple of 256B (encoded as stride/256 in the descriptor)."""
    import concourse.ap_utils as ap_utils
    from concourse.bass import MemorySpace

    assert idxs_ap.dtype == i16
    assert in_ap.space == MemorySpace.DRAM
    assert idxs_ap.space == MemorySpace.SBUF
    assert out_ap.space == MemorySpace.SBUF
    assert ap_utils.ap_is_contiguous(out_ap.ap[1:])
    assert ap_utils.ap_is_contiguous(idxs_ap.ap[1:])
    assert in_ap.ap[-1][1] == out_ap.ap[-1][1] == elem_size
    assert out_ap.ap[0][1] * out_ap.ap[1][1] == ((num_idxs + 127) // 128) * 128
    assert in_ap.ap[0][0] == elem_step
    stride_bytes = elem_step * mybir.dt.size(in_ap.dtype)
    stride_bytes_256 = stride_bytes // 256
    assert stride_bytes % 256 == 0 and stride_bytes_256 < 256
    _in_ap = gp.lower_ap_dma(in_ap, for_custom_bir_dma=True)
    _idxs_ap = gp.lower_ap(idxs_ap)
    _out_ap = gp.lower_ap(out_ap)
    return gp.add_instruction(
        mybir.InstDMAGatherAnt(
            name=gp.bass.get_next_instruction_name(),
            ins=[*_in_ap, _idxs_ap, gp.lower_val_access(gp.to_reg(num_idxs))],
            outs=[_out_ap],
            transpose=False,
            num_idxs=num_idxs,
            elem_size=elem_size,
            stride_bytes_256=stride_bytes_256,
            gen_mode=0,
            single_packet=True,
            queue_num=0,
            sbuf_tokens_per_rank=0,
            sbuf_free_dim_per_rank=0,
            sbuf_free_dim_pad_per_rank=0,
            sbuf_byte_offset=0,
        )
    )


# ------------------------------------------------------------- device build

def _build(dims, single=False):
    npad, nloc, bpc = dims["npad"], dims["nloc"], dims["bpc"]
    cpt, ch = dims["cpt"], dims["ch"]
    hi_base = dims["hi_base"]
    use_bias = dims.get("use_bias", True)

    def group_calls(calls):
        by_tile = [[] for _ in range(dims["bpc"])]
        for side, c0, J in calls:
            by_tile[c0 // dims["cpt"]].append((side, c0, J))
        return by_tile

    calls_by_tile_big = group_calls(dims["calls"])
    calls_by_tile_l2 = group_calls(dims["calls2"])

    nc = bacc.Bacc("TRN2", target_bir_lowering=False, debug=False,
                   enable_asserts=True,
                   dynamic_dma_scratch_size=49152,
                   num_devices=1 if single else NCORES)

    # inputs
    xt_ap = nc.dram_tensor("xt", [D0, nloc], bf16, kind="ExternalInput").ap()
    srcidx_ap = nc.dram_tensor("srcidx", [P, ch * 8], i16,
                               kind="ExternalInput").ap()
    pmpt_ap = nc.dram_tensor("pmpt", [P, ch * 2 * P], fp8,
                             kind="ExternalInput").ap()
    w0_ap = nc.dram_tensor("w0", [D0, HD], bf16, kind="ExternalInput").ap()
    w1_ap = nc.dram_tensor("w1", [HD, HD], bf16, kind="ExternalInput").ap()
    w2e_ap = nc.dram_tensor("w2e", [HD, COL2], bf16, kind="ExternalInput").ap()
    w0a_ap = nc.dram_tensor("w0a", [D0, 2 * NH], bf16, kind="ExternalInput").ap()
    w1a_ap = nc.dram_tensor("w1a", [HD, 2 * NH], bf16, kind="ExternalInput").ap()
    b0_ap = nc.dram_tensor("b0r", [1, HD], bf16, kind="ExternalInput").ap()
    b1_ap = nc.dram_tensor("b1r", [1, HD], bf16, kind="ExternalInput").ap()
    b2_ap = nc.dram_tensor("b2r", [1, OUTC], bf16, kind="ExternalInput").ap()
    out_ap = nc.dram_tensor("out", [nloc, OUTC], f32, kind="ExternalOutput").ap()

    with tile.TileContext(nc) as tc:
        with tc.tile_pool(name="const", bufs=1) as cp, \
             tc.tile_pool(name="work", bufs=2) as sb, \
             tc.tile_pool(name="psum", bufs=1, space="PSUM") as ps, \
             tc.tile_pool(name="dram", bufs=1, space="DRAM") as dp:

            # ---------- persistent constants in SBUF
            ident_t = cp.tile([P, P], bf16)
            make_identity(nc, ident_t[:])
            ones_t = cp.tile([1, P], bf16)
            nc.gpsimd.memset(ones_t[:], 1.0)
            ones128_t = cp.tile([P, 1], bf16)
            nc.gpsimd.memset(ones128_t[:], 1.0)

            xt_t = cp.tile([P, nloc], bf16)
            nc.sync.dma_start(xt_t[:], xt_ap[:])
            idxall_t = cp.tile([P, ch * 8], i16)
            nc.sync.dma_start(idxall_t[:], srcidx_ap[:])
            w0_t = cp.tile([P, HD], bf16)
            nc.sync.dma_start(w0_t[:], w0_ap[:])
            w0a_t = cp.tile([P, 2 * NH], bf16)
            nc.sync.dma_start(w0a_t[:], w0a_ap[:])
            w1_t = [cp.tile([P, HD], bf16, name=f"w1c{k}", tag=f"w1_{k}")
                    for k in range(4)]
            w1a_t = [cp.tile([P, 2 * NH], bf16, name=f"w1ac{k}", tag=f"w1a_{k}")
                     for k in range(4)]
            w2e_t = [cp.tile([P, COL2], bf16, name=f"w2ec{k}", tag=f"w2e_{k}")
                     for k in range(4)]
            for k in range(4):
                nc.sync.dma_start(w1_t[k][:], w1_ap[k * P:(k + 1) * P, :])
                nc.sync.dma_start(w1a_t[k][:], w1a_ap[k * P:(k + 1) * P, :])
                nc.sync.dma_start(w2e_t[k][:], w2e_ap[k * P:(k + 1) * P, :])

            # a_dst tables, kept SBUF-resident per layer
            adst0_t = cp.tile([P, bpc * NH], bf16)
            adst1_t = cp.tile([P, bpc * NH], bf16)
            adst2_t = cp.tile([P, bpc], bf16)

            # bias tiles broadcast across partitions via K=1 matmul
            def bias_tile(b_ap, width, dt, tag):
                row = sb.tile([1, width], bf16, tag="brow")
                nc.sync.dma_start(row[:], b_ap[:])
                bps = ps.tile([P, width], f32, tag="h", bufs=1)
                nc.tensor.matmul(out=bps[:], lhsT=ones_t[:], rhs=row[:],
                                 start=True, stop=True)
                bt = cp.tile([P, width], dt, name=tag, tag=tag)
                nc.scalar.activation(bt[:], bps[:], AF.Copy)
                return bt

            if use_bias:
                b0_t = bias_tile(b0_ap, HD, bf16, "b0t")
                b1_t = bias_tile(b1_ap, HD, bf16, "b1t")
                b2_t = bias_tile(b2_ap, OUTC, f32, "b2t")
            else:
                b0_t = b1_t = b2_t = None

            # ---------- DRAM scratch
            ag0_in = dp.tile([nloc, ROW0], bf16)
            ag1_in = dp.tile([nloc, ROW0], bf16)
            ag2_in = dp.tile([nloc, ROW2], bf16)
            hcat0 = dp.tile([npad, ROW0], bf16)
            hcat1 = dp.tile([npad, ROW0], bf16)
            hcat2 = dp.tile([npad, ROW2], bf16)

            # ---------- helpers
            def assemble_big(t, h_ps, a_ps, ag_in, adst_next):
                """Write packed row [h|asrc] for tile t and stash a_dst."""
                cat = sb.tile([P, COL0], bf16, tag="cat", bufs=3)
                nc.scalar.activation(cat[:, :HD], h_ps[:], AF.Copy)
                nc.scalar.activation(cat[:, HD:COL0], a_ps[:, :NH], AF.Copy)
                nc.scalar.activation(adst_next[:, t * NH:(t + 1) * NH],
                                     a_ps[:, NH:2 * NH], AF.Copy)
                nc.scalar.dma_start(ag_in[t * P:(t + 1) * P, :COL0], cat[:])

            def node_phase_l0():
                for t in range(bpc):
                    h_ps = ps.tile([P, HD], f32, tag="h", bufs=1)
                    a_ps = ps.tile([P, 2 * NH], f32, tag="a", bufs=1)
                    lhs = xt_t[:, t * P:(t + 1) * P]
                    nc.tensor.matmul(out=h_ps[:], lhsT=lhs, rhs=w0_t[:],
                                     start=True, stop=True)
                    nc.tensor.matmul(out=a_ps[:], lhsT=lhs, rhs=w0a_t[:],
                                     start=True, stop=True)
                    assemble_big(t, h_ps, a_ps, ag0_in, adst0_t)

            def allgather(ag_in, hcat_f):
                if single:
                    # timeline-sim mode: stand-in copy, no collective;
                    # split so the copy overlaps the producing flushes
                    bounds = [k * P for k in range(bpc)] + [nloc]
                    for r0, r1 in zip(bounds[:-1], bounds[1:]):
                        nc.sync.dma_start(hcat_f[r0:r1, :],
                                          ag_in[r0:r1, :])
                else:
                    nc.gpsimd.collective_compute(
                        "AllGather", ALU.bypass,
                        replica_groups=[list(range(NCORES))],
                        ins=[ag_in[:].opt()],
                        outs=[hcat_f[:npad, :].opt()],
                    )

            def flush_big(t, o_ps, d_ps, b_t, w_next, wa_next, ag_next,
                          adst_next, last):
                """Normalize tile t, relu, then fused next-layer node phase."""
                den = sb.tile([P, NH], f32, tag="den_sb")
                nc.vector.tensor_scalar_add(den[:], d_ps[:], 1e-20)
                rden = sb.tile([P, NH], f32, tag="rden")
                nc.vector.reciprocal(rden[:], den[:])
                o_sb = sb.tile([P, HD], bf16, tag="osb")
                nc.vector.tensor_tensor(
                    out=o_sb[:].rearrange("p (h c) -> p h c", c=HC),
                    in0=o_ps[:].rearrange("p (h c) -> p h c", c=HC),
                    in1=rden[:].rearrange("p (h one) -> p h one", one=1)
                        .broadcast_to([P, NH, HC]),
                    op=ALU.mult)
                if use_bias:
                    nc.vector.tensor_tensor(out=o_sb[:], in0=o_sb[:],
                                            in1=b_t[:], op=ALU.add)
                act = sb.tile([P, HD], bf16, tag="act")
                nc.scalar.activation(act[:], o_sb[:], AF.Relu)
                if last:
                    h_ps = ps.tile([P, COL2], f32, tag="h", bufs=1)
                else:
                    h_ps = ps.tile([P, HD], f32, tag="h", bufs=1)
                    a_ps = ps.tile([P, 2 * NH], f32, tag="a", bufs=1)
                for k in range(4):
                    tr_ps = ps.tile([P, P], bf16, tag="dtr", bufs=2)
                    nc.tensor.transpose(out=tr_ps[:],
                                        in_=act[:, k * P:(k + 1) * P],
                                        identity=ident_t[:])
                    trsb = sb.tile([P, P], bf16, tag="trsb", bufs=3)
                    nc.scalar.activation(trsb[:], tr_ps[:], AF.Copy)
                    nc.tensor.matmul(out=h_ps[:], lhsT=trsb[:],
                                     rhs=w_next[k][:],
                                     start=(k == 0), stop=(k == 3))
                    if not last:
                        nc.tensor.matmul(out=a_ps[:], lhsT=trsb[:],
                                         rhs=wa_next[k][:],
                                         start=(k == 0), stop=(k == 3))
                if last:
                    cat = sb.tile([P, COL2], bf16, tag="cat2")
                    nc.scalar.activation(cat[:, :OUTC], h_ps[:, :OUTC],
                                         AF.Copy)
                    nc.scalar.activation(cat[:, OUTC:OUTC + 1],
                                         ones128_t[:], AF.Copy)
                    nc.scalar.activation(cat[:, OUTC + 1:OUTC + 2],
                                         h_ps[:, OUTC + 1:OUTC + 2], AF.Copy)
                    nc.scalar.activation(adst_next[:, t:t + 1],
                                         h_ps[:, OUTC + 2:OUTC + 3], AF.Copy)
                    nc.scalar.dma_start(
                        ag_next[t * P:(t + 1) * P, :COL2 - 1], cat[:, :COL2 - 1])
                else:
                    assemble_big(t, h_ps, a_ps, ag_next, adst_next)

            def flush_l2(t, o_ps, d_ps):
                den = sb.tile([P, 1], f32, tag="den_sb")
                nc.vector.tensor_scalar_add(den[:], o_ps[:, OUTC:OUTC + 1],
                                            1e-20)
                rden = sb.tile([P, 1], f32, tag="rden")
                nc.vector.reciprocal(rden[:], den[:])
                o_sb = sb.tile([P, OUTC], f32, tag="osb2")
                nc.vector.tensor_tensor(out=o_sb[:], in0=o_ps[:, :OUTC],
                                        in1=rden[:].broadcast_to([P, OUTC]),
                                        op=ALU.mult)
                if use_bias:
                    nc.vector.tensor_tensor(out=o_sb[:], in0=o_sb[:],
                                            in1=b2_t[:], op=ALU.add)
                mx = sb.tile([P, 1], f32, tag="mx")
                nc.vector.tensor_reduce(out=mx[:], in_=o_sb[:],
                                        axis=mybir.AxisListType.X, op=ALU.max)
                t2 = sb.tile([P, OUTC], f32, tag="t2")
                nc.vector.tensor_tensor(out=t2[:], in0=o_sb[:],
                                        in1=mx[:].broadcast_to([P, OUTC]),
                                        op=ALU.subtract)
                exl = sb.tile([P, OUTC], f32, tag="exl")
                sm = sb.tile([P, 1], f32, tag="sm")
                nc.scalar.activation(exl[:], t2[:], AF.Exp, accum_out=sm[:])
                ls = sb.tile([P, 1], f32, tag="ls")
                nc.scalar.activation(ls[:], sm[:], AF.Ln)
                res = sb.tile([P, OUTC], f32, tag="res")
                nc.vector.tensor_tensor(out=res[:], in0=t2[:],
                                        in1=ls[:].broadcast_to([P, OUTC]),
                                        op=ALU.subtract)
                nc.scalar.dma_start(out_ap[t * P:(t + 1) * P, :], res[:])

            def agg_layer(hcat_f, roww, hwid, nhh, adst_t, flush):
                """Edge aggregation for one layer.

                For layers 0/1 (nhh=NH) the denominators come from a second
                N=8 matmul per chunk. For layer 2 (nhh=1) the table carries a
                constant-1 channel at column OUTC, so the single scatter
                matmul also accumulates the denominator (hwid includes it).
                """
                fold_den = nhh != NH
                calls_by_tile = (calls_by_tile_l2 if fold_den
                                 else calls_by_tile_big)
                asrc_c = hwid
                payw = hwid + nhh          # gathered cols: mult region + asrc
                lo_view = hcat_f[0:32768, :payw]
                hi_view = hcat_f[hi_base:, :payw]
                state = {}
                for tt in range(bpc):
                    pmpt_sb = sb.tile([P, cpt * 2 * P], fp8, tag="pmpt", bufs=4)
                    nc.sync.dma_start(
                        pmpt_sb[:],
                        pmpt_ap[:, tt * cpt * 2 * P:(tt + 1) * cpt * 2 * P])

                    # a_dst per edge for the whole tile, hoisted
                    ade_ps = ps.tile([P, cpt * nhh], f32, tag="ade", bufs=2)
                    for jj in range(cpt):
                        nc.tensor.matmul(
                            out=ade_ps[:, jj * nhh:(jj + 1) * nhh],
                            lhsT=pmpt_sb[:, jj * 2 * P + P:(jj + 1) * 2 * P],
                            rhs=adst_t[:, tt * nhh:(tt + 1) * nhh],
                            start=True, stop=True)

                    for side, c0, J in calls_by_tile[tt]:
                        jj0 = c0 - tt * cpt       # local chunk offset in tile

                        rowg = sb.tile([P, 8 * payw], bf16, tag="rowg",
                                       bufs=8)
                        _dma_gather_raw(
                            nc.gpsimd,
                            rowg[:, :J * payw].rearrange(
                                "p (j w) -> p j w", w=payw),
                            lo_view if side == 0 else hi_view,
                            idxall_t[:, c0 * 8:(c0 + J) * 8],
                            J * P, payw, roww)

                        # logits; ex = exp(leaky_relu(l)) = max(e^l, e^0.2l)
                        ne = J * nhh
                        rview = rowg[:, :J * payw].rearrange(
                            "p (j w) -> p j w", j=J)
                        lg = sb.tile([P, 8 * nhh], f32, tag="lg", bufs=4)
                        nc.vector.tensor_tensor(
                            out=lg[:, :ne].rearrange("p (j h) -> p j h",
                                                     j=J),
                            in0=rview[:, :, asrc_c:asrc_c + nhh],
                            in1=ade_ps[:, jj0 * nhh:(jj0 + J) * nhh]
                                .rearrange("p (j h) -> p j h", j=J),
                            op=ALU.add)
                        e1 = sb.tile([P, 8 * nhh], bf16, tag="e1", bufs=4)
                        nc.scalar.activation(e1[:, :ne], lg[:, :ne], AF.Exp)
                        e2 = sb.tile([P, 8 * nhh], bf16, tag="e2", bufs=4)
                        nc.scalar.activation(e2[:, :ne], lg[:, :ne], AF.Exp,
                                             scale=NEG)
                        # sc = rows * ex, one DVE op per call
                        sc = sb.tile([P, 8 * hwid], bf16, tag="sc", bufs=4)
                        if not fold_den:
                            # ex replicated x4, fused with the leaky-relu max
                            exr = sb.tile([P, 8 * NH * 4], bf16, tag="exr", bufs=4)
                            nc.vector.tensor_tensor(
                                out=exr[:, :ne * 4].rearrange(
                                    "p (e b) -> p e b", b=4),
                                in0=e1[:, :ne].rearrange(
                                    "p (e one) -> p e one", one=1)
                                    .broadcast_to([P, ne, 4]),
                                in1=e2[:, :ne].rearrange(
                                    "p (e one) -> p e one", one=1)
                                    .broadcast_to([P, ne, 4]),
                                op=ALU.max)
                            nc.vector.tensor_tensor(
                                out=sc[:, :J * hwid].rearrange(
                                    "p (j h a b) -> p j h a b",
                                    j=J, h=NH, a=16, b=4),
                                in0=rview[:, :, :hwid].rearrange(
                                    "p j (h a b) -> p j h a b",
                                    h=NH, a=16, b=4),
                                in1=exr[:, :ne * 4].rearrange(
                                    "p (j h one b) -> p j h one b",
                                    j=J, one=1, b=4)
                                    .broadcast_to([P, J, NH, 16, 4]),
                                op=ALU.mult)
                        else:
                            ex = sb.tile([P, 8 * nhh], bf16, tag="ex",
                                         bufs=3)
                            nc.vector.tensor_tensor(out=ex[:, :ne],
                                                    in0=e1[:, :ne],
                                                    in1=e2[:, :ne],
                                                    op=ALU.max)
                            nc.vector.tensor_tensor(
                                out=sc[:, :J * hwid].rearrange(
                                    "p (j w) -> p j w", j=J),
                                in0=rview[:, :, :hwid],
                                in1=ex[:, :ne].rearrange(
                                    "p (j one) -> p j one", one=1)
                                    .broadcast_to([P, J, hwid]),
                                op=ALU.mult)

                        # scatter matmuls, accumulated per destination tile
                        for j in range(J):
                            chk = c0 + j
                            t, c = divmod(chk, cpt)
                            if c == 0:
                                o_ps = ps.tile([P, hwid], f32, tag="o",
                                               name="o_ps", bufs=2)
                                if not fold_den:
                                    d_ps = ps.tile([P, nhh], f32, tag="dtr",
                                                   name="d_ps", bufs=2)
                                else:
                                    d_ps = None
                                state[t] = (o_ps, d_ps)
                            o_ps, d_ps = state[t]
                            if not fold_den:
                                nc.tensor.matmul(
                                    out=d_ps[:],
                                    lhsT=pmpt_sb[:, (jj0 + j) * 2 * P:
                                                 (jj0 + j) * 2 * P + P],
                                    rhs=exr[:, j * NH * 4:(j + 1) * NH * 4]
                                        .rearrange("p (h b) -> p h b", b=4)
                                        [:, :, 0:1]
                                        .rearrange("p h one -> p (h one)"),
                                    start=(c == 0), stop=(c == cpt - 1))
                            nc.tensor.matmul(
                                out=o_ps[:],
                                lhsT=pmpt_sb[:, (jj0 + j) * 2 * P:
                                             (jj0 + j) * 2 * P + P],
                                rhs=sc[:, j * hwid:(j + 1) * hwid],
                                start=(c == 0), stop=(c == cpt - 1))
                            if c == cpt - 1:
                                flush(t, o_ps, d_ps)
                                del state[t]

            # ---------- the program
            node_phase_l0()
            allgather(ag0_in, hcat0)
            agg_layer(hcat0, ROW0, HD, NH, adst0_t,
                      lambda t, o, d: flush_big(t, o, d, b0_t, w1_t, w1a_t,
                                                ag1_in, adst1_t, False))
            allgather(ag1_in, hcat1)
            agg_layer(hcat1, ROW0, HD, NH, adst1_t,
                      lambda t, o, d: flush_big(t, o, d, b1_t, w2e_t, None,
                                                ag2_in, adst2_t, True))
            allgather(ag2_in, hcat2)
            agg_layer(hcat2, ROW2, OUTC + 1, 1, adst2_t, flush_l2)

    nc.compile()
    return nc


# ------------------------------------------------------------------ runners

_CACHE = {}


def _get_program(dims):
    key = tuple(sorted(dims.items()))
    if key not in _CACHE:
        _CACHE[key] = _build(dims)
    return _CACHE[key]


def make_in_maps(x, W0, as0, ad0, b0, W1, as1, ad1, b1, W2, as2, ad2, b2,
                 dims, per_core, new_id):
    npad, nloc = dims["npad"], dims["nloc"]
    xp = np.zeros((npad, D0), np.float32)
    xp[new_id] = np.asarray(x, np.float32)

    W0 = np.asarray(W0, np.float32)
    W1 = np.asarray(W1, np.float32)
    W2 = np.asarray(W2, np.float32)
    # layer-2 fused weights: [W2 | W2@as2 | W2@ad2]
    w2e = np.zeros((HD, COL2), np.float32)
    w2e[:, :OUTC] = W2
    w2e[:, OUTC + 1] = W2 @ np.asarray(as2, np.float32)[0]
    w2e[:, OUTC + 2] = W2 @ np.asarray(ad2, np.float32)[0]

    w0a = W0 @ _block_diag_a(np.asarray(as0, np.float32),
                             np.asarray(ad0, np.float32))
    w1a = W1 @ _block_diag_a(np.asarray(as1, np.float32),
                             np.asarray(ad1, np.float32))

    shared = {
        "w0": W0.astype(npbf16),
        "w1": W1.astype(npbf16),
        "w2e": w2e.astype(npbf16),
        "w0a": w0a.astype(npbf16),
        "w1a": w1a.astype(npbf16),
        "b0r": np.asarray(b0, np.float32).reshape(1, HD).astype(npbf16),
        "b1r": np.asarray(b1, np.float32).reshape(1, HD).astype(npbf16),
        "b2r": np.asarray(b2, np.float32).reshape(1, OUTC).astype(npbf16),
    }
    in_maps = []
    for c in range(NCORES):
        m = dict(shared)
        m["xt"] = np.ascontiguousarray(
            xp[c * nloc:(c + 1) * nloc].T).astype(npbf16)
        m.update(per_core[c])
        in_maps.append(m)
    return in_maps


def assemble_output(results, dims, new_id):
    n = dims["n"]
    full = np.concatenate([results[c]["out"] for c in range(NCORES)], axis=0)
    return np.ascontiguousarray(full[new_id[:n]])


def kernel(x, edge_index, W0, as0, ad0, b0, W1, as1, ad1, b1,
           W2, as2, ad2, b2):
    from concourse import bass_utils

    n = x.shape[0]
    dims, per_core, new_id = _prep(np.asarray(edge_index), n)
    dims = _with_bias_flag(dims, b0, b1, b2)
    prog = _get_program(dims)
    in_maps = make_in_maps(x, W0, as0, ad0, b0, W1, as1, ad1, b1,
                           W2, as2, ad2, b2, dims, per_core, new_id)
    res = bass_utils.run_bass_kernel_spmd(prog, in_maps,
                                          core_ids=list(range(NCORES)))
    return assemble_output(res.results, dims, new_id)
